# revision 2
# baseline (speedup 1.0000x reference)
"""Trainium2 kernel for nn_LinearMem: bit-sliced int8-quantized linear layer.

Math: the reference's 16 bit-plane matmuls recombine exactly to
qx @ qw^T with qx = round(x/sx), qw = round(w/sw); see kernel_baseline.py.
Products/partials are integers < 2^24, so bf16 x bf16 matmul with f32 PSUM
reproduces it (int8 exact in bf16).

v2 schedule (per core; 2x4 tensor-parallel grid as before):
  - x ships HOST-PRESCALED: bf16(s * qx^T) per m-tile on DMA ring B
    (Scalar-engine triggers).  No on-device x cast, no scale tensor;
    the bf16 rounding of s*qx adds ~1e-3 global rel err (gate is 2e-2).
  - w ships int8 on ring A (Sync triggers) in size-ramped k-chunks
    (1,1,2,4,4,4 k-blocks); DVE casts each 1-2-k-block group to bf16 as
    it lands, so the first real matmul issues at ~9 us instead of ~14.
  - 4 warmup matmuls (vs 19) release the HAM clock gate; the residual
    1.2 GHz ramp hides inside the w-cast pacing of m-tile 0.
  - 8 m-tiles x 16 k matmuls accumulate in 4 rotating PSUM banks.
  - output: per m-tile, (acc + bias) in two 256-col DVE passes, each
    immediately DMA'd on alternating rings; last tile in 4 128-col
    chunks so the final receipt chain is short.
"""

import sys

if "/opt/trn_rl_repo" not in sys.path:
    sys.path.insert(0, "/opt/trn_rl_repo")

import ml_dtypes
import numpy as np

import concourse.bacc as bacc
import concourse.mybir as mybir
import concourse.tile as tile
from concourse.bass_utils import run_bass_kernel_spmd

M, K, N = 2048, 2048, 2048
PM, PN = 2, 4  # grid: M split PM ways, N split PN ways
MS, NS = M // PM, N // PN  # per-core shard sizes: 1024, 512
MT = MS // 128  # 8 m-tiles
KT = K // 128  # 16 k-blocks

F32 = mybir.dt.float32
BF16 = mybir.dt.bfloat16
I8 = mybir.dt.int8

# w DMA chunk sizes in k-blocks (ramped so the first cast lands early)
WCHUNKS = (1, 1, 2, 4, 4, 4)
# w cast groups in k-blocks (each one DVE tensor_copy, own tile)
WCAST = (1, 1, 2, 2, 2, 2, 2, 2, 2)
N_WARM = 4


def _build_program():
    nc = bacc.Bacc("TRN2", target_bir_lowering=False, debug=False, num_devices=8)

    wch_off = np.concatenate([[0], np.cumsum(WCHUNKS)])
    wca_off = np.concatenate([[0], np.cumsum(WCAST)])
    assert wch_off[-1] == KT and wca_off[-1] == KT

    # int8 w in k-lane-major order [k%128, k//128, n]; x prescaled bf16
    # per m-tile [k%128, k//128, m%128]; both partition-contiguous.
    qw_in = nc.dram_tensor("qwt_sh", [128, KT, NS], I8, kind="ExternalInput")
    xs_in = nc.dram_tensor("xsc_sh", [MT, 128, KT, 128], BF16, kind="ExternalInput")
    b_in = nc.dram_tensor("b_sh", [1, NS], F32, kind="ExternalInput")
    out_t = nc.dram_tensor("out_sh", [MS, NS], F32, kind="ExternalOutput")

    with tile.TileContext(nc) as tc:
        with (
            tc.tile_pool(name="const", bufs=1) as const,
            tc.tile_pool(name="w8p", bufs=1) as w8p,
            tc.tile_pool(name="wbp", bufs=1) as wbp,
            tc.tile_pool(name="xp", bufs=1) as xp,
            tc.tile_pool(name="out", bufs=3) as op,
            tc.tile_pool(name="psum", bufs=4, space="PSUM") as ps,
            tc.tile_pool(name="warm", bufs=1, space="PSUM") as wm,
        ):
            # PE warmup source: nonzero bf16 (zero MACs are power-gated and
            # don't release the HAM clock gate).  memset on gpsimd so the
            # Tensor queue isn't gated on the Vector preamble.
            zsrc = const.tile([128, NS], BF16, tag="zsrc")
            nc.gpsimd.memset(zsrc[:], 1.0)
            zacc = wm.tile([128, NS], F32, tag="zacc", name="zacc")
            for _ in range(N_WARM):
                nc.tensor.matmul(zacc[:], zsrc[:, 0:128], zsrc[:], start=True, stop=True)

            # ring A (sync): w chunks, ramped sizes
            w8 = []
            for ci, nkb in enumerate(WCHUNKS):
                t = w8p.tile([128, nkb, NS], I8, tag=f"w8_{ci}", name=f"w8_{ci}")
                w8.append(t)
                nc.sync.dma_start(t[:], qw_in[:, wch_off[ci] : wch_off[ci + 1], :])

            # ring B (scalar): x m-tiles; m0 in two halves for early start
            xm = []
            xm0a = xp.tile([128, 8, 128], BF16, tag="x0a", name="x0a")
            xm0b = xp.tile([128, 8, 128], BF16, tag="x0b", name="x0b")
            nc.scalar.dma_start(xm0a[:], xs_in[0][:, 0:8, :])
            nc.scalar.dma_start(xm0b[:], xs_in[0][:, 8:16, :])
            for m in range(1, MT):
                t = xp.tile([128, KT, 128], BF16, tag=f"x{m}", name=f"x{m}")
                xm.append(t)
                nc.scalar.dma_start(t[:], xs_in[m])

            # bias via SWDGE + broadcast (gpsimd, off the critical path)
            bias_row = const.tile([1, NS], F32, tag="bias_row")
            nc.gpsimd.dma_start(bias_row[:], b_in[:])
            bias_b = const.tile([128, NS], F32, tag="bias_b")
            nc.gpsimd.partition_broadcast(bias_b[:], bias_row[:], channels=128)

            # w casts: one DVE tensor_copy per 1-2-k-block group, in k order
            wb = []
            for gi, gkb in enumerate(WCAST):
                t = wbp.tile([128, gkb, NS], BF16, tag=f"wb_{gi}", name=f"wb_{gi}")
                wb.append(t)
                # source chunk(s) covering [wca_off[gi], wca_off[gi+1])
                k0, k1 = wca_off[gi], wca_off[gi + 1]
                # find chunk containing k0 (cast groups never straddle chunks
                # given WCHUNKS/WCAST above)
                ci = int(np.searchsorted(wch_off, k0, side="right")) - 1
                assert k1 <= wch_off[ci + 1]
                s0 = k0 - wch_off[ci]
                nc.vector.tensor_copy(t[:], w8[ci][:, s0 : s0 + gkb, :])

            def wb_ap(kb):
                gi = int(np.searchsorted(wca_off, kb, side="right")) - 1
                return wb[gi][:, kb - wca_off[gi], :]

            def x_ap(mb, kb):
                if mb == 0:
                    return (xm0a if kb < 8 else xm0b)[:, kb % 8, :]
                return xm[mb - 1][:, kb, :]

            for mb in range(MT):
                acc = ps.tile([128, NS], F32, tag="acc")
                for kb in range(KT):
                    nc.tensor.matmul(
                        acc[:],
                        x_ap(mb, kb),
                        wb_ap(kb),
                        start=(kb == 0),
                        stop=(kb == KT - 1),
                    )
                # (acc + bias) -> SBUF in column chunks, each chunk DMA'd
                # immediately; rings alternate to split the receipt chain.
                o2 = op.tile([128, NS], F32, tag="o2")
                rows = out_t[mb * 128 : (mb + 1) * 128, :]
                nch = 4 if mb == MT - 1 else 2
                cw = NS // nch
                for c in range(nch):
                    sl = slice(c * cw, (c + 1) * cw)
                    nc.vector.scalar_tensor_tensor(
                        o2[:, sl], acc[:, sl], 1.0, bias_b[:, sl],
                        op0=mybir.AluOpType.mult, op1=mybir.AluOpType.add,
                    )
                    eng = nc.sync if c % 2 == 0 else nc.scalar
                    eng.dma_start(rows[:, sl], o2[:, sl])

    nc.compile()
    return nc


_NC = None


def _get_nc():
    global _NC
    if _NC is None:
        _NC = _build_program()
    return _NC


def _quantize(a):
    """Exactly the reference's quantization: scale = amax/127 (f32 IEEE),
    q = clip(round-half-even(a / scale), -127, 127)."""
    amax = np.float32(np.max(np.abs(a)))
    scale = amax / np.float32(127.0)
    q = np.clip(np.round((a / scale).astype(np.float32)), -127.0, 127.0)
    return q.astype(np.int8), scale


def kernel(x, weight, bias, _trace=False):
    x = np.asarray(x, dtype=np.float32)
    weight = np.asarray(weight, dtype=np.float32)
    bias = np.asarray(bias, dtype=np.float32)

    qx, sx = _quantize(x)
    qw, sw = _quantize(weight)
    s = np.float32(sx * sw)

    # x^T, prescaled by s, in bf16, m-tile-major [MT,128,KT,128] per core
    xsc = (qx.T.astype(np.float32) * s).astype(ml_dtypes.bfloat16)  # [K, M]
    qwt = qw.T  # [K, N]

    in_maps = []
    for c in range(8):
        i, j = divmod(c, PN)
        xs = xsc[:, i * MS : (i + 1) * MS]  # [K, MS]
        xs = np.ascontiguousarray(
            xs.reshape(KT, 128, MT, 128).transpose(2, 1, 0, 3)
        )  # [MT, 128, KT, 128]
        ws = qwt[:, j * NS : (j + 1) * NS]  # [K, NS]
        ws = np.ascontiguousarray(
            ws.reshape(KT, 128, NS).transpose(1, 0, 2)
        )  # [128, KT, NS]
        in_maps.append(
            {
                "xsc_sh": xs,
                "qwt_sh": ws,
                "b_sh": bias[j * NS : (j + 1) * NS].reshape(1, NS),
            }
        )

    nc = _get_nc()
    try:
        res = run_bass_kernel_spmd(nc, in_maps, core_ids=list(range(8)), trace=_trace)
    except Exception:
        # rare transient NRT device hiccups recover on retry
        res = run_bass_kernel_spmd(nc, in_maps, core_ids=list(range(8)), trace=_trace)

    out = np.empty((M, N), np.float32)
    for c in range(8):
        i, j = divmod(c, PN)
        out[i * MS : (i + 1) * MS, j * NS : (j + 1) * NS] = res.results[c]["out_sh"]
    if _trace:
        return out, res
    return out


# revision 3
# speedup vs baseline: 1.0622x; 1.0622x over previous
"""Trainium2 kernel for nn_LinearMem: bit-sliced int8-quantized linear layer.

Math: the reference's 16 bit-plane matmuls recombine exactly to
qx @ qw^T with qx = round(x/sx), qw = round(w/sw).  Products/partials are
integers < 2^24, so bf16 x bf16 matmuls with f32 PSUM accumulation
reproduce the reference bitwise (int8 values are exact in bf16).

v3 schedule (per core; 2x4 tensor-parallel grid):
  - Input pipe analysis: one HWDGE ring sustains ~273 B/ns and the
    int8->bf16 cast engines ~219 (DVE) / ~125 (ACT) elem/ns, so m-tile 0
    (which consumes ALL of w) is paced by w delivery.  Fix: w k-blocks
    0-7 ship int8 on ring A and are DVE-cast; k-blocks 8-15 ship as
    bf16 (exact, values <= 127) on the otherwise-idle ring B, needing
    no cast and no ring-A bandwidth.
  - Ring A (sync): ramped w chunks [1,1,2,2,2] kb interleaved with the
    first x half-tiles, then x m1..m7 (int8, ACT+DVE cast halves).
  - 5 warmup matmuls release the HAM clock gate; real matmuls start
    ~10 us, m-tile 0 is input-paced to ~13.5 us, m1-m7 run at the
    216 ns/matmul roofline.
  - Output: per m-tile (acc*s + bias) in two 256-col DVE passes, each
    DMA'd immediately on ring B; the last tile uses four 128-col
    chunks on alternating rings so the final receipt chain is short.
"""

import sys

if "/opt/trn_rl_repo" not in sys.path:
    sys.path.insert(0, "/opt/trn_rl_repo")

import ml_dtypes
import numpy as np

import concourse.bacc as bacc
import concourse.mybir as mybir
import concourse.tile as tile
from concourse.bass_utils import run_bass_kernel_spmd

M, K, N = 2048, 2048, 2048
PM, PN = 2, 4
MS, NS = M // PM, N // PN  # 1024, 512
MT = MS // 128  # 8
KT = K // 128  # 16
KLO = 8  # k-blocks shipped int8 on ring A (DVE-cast); rest bf16 on ring B

F32 = mybir.dt.float32
BF16 = mybir.dt.bfloat16
I8 = mybir.dt.int8

# ring-A w chunk sizes in k-blocks (kb 0..7), ramped for an early first cast
WCHUNKS = (1, 1, 2, 2, 2)
N_WARM = 5


def _build_program():
    nc = bacc.Bacc("TRN2", target_bir_lowering=False, debug=False, num_devices=8)

    wch_off = np.concatenate([[0], np.cumsum(WCHUNKS)])
    assert wch_off[-1] == KLO

    qw_in = nc.dram_tensor("qw_lo", [128, KLO, NS], I8, kind="ExternalInput")
    wh_in = nc.dram_tensor("w_hi", [128, KT - KLO, NS], BF16, kind="ExternalInput")
    qx_in = nc.dram_tensor("qxt_sh", [MT, 128, KT, 128], I8, kind="ExternalInput")
    b_in = nc.dram_tensor("b_sh", [1, NS], F32, kind="ExternalInput")
    scl_in = nc.dram_tensor("scl", [1, 4], F32, kind="ExternalInput")
    out_t = nc.dram_tensor("out_sh", [MS, NS], F32, kind="ExternalOutput")

    with tile.TileContext(nc) as tc:
        with (
            tc.tile_pool(name="const", bufs=1) as const,
            tc.tile_pool(name="w8p", bufs=1) as w8p,
            tc.tile_pool(name="wbp", bufs=1) as wbp,
            tc.tile_pool(name="x8p", bufs=1) as x8p,
            tc.tile_pool(name="xbp", bufs=1) as xbp,
            tc.tile_pool(name="out", bufs=3) as op,
            tc.tile_pool(name="psum", bufs=4, space="PSUM") as ps,
            tc.tile_pool(name="warm", bufs=1, space="PSUM") as wm,
        ):
            # PE warmup: nonzero bf16 source (zero MACs are power-gated and
            # don't count as HAM activity); memset on gpsimd so the Tensor
            # queue isn't gated on the Vector preamble.
            zsrc = const.tile([128, NS], BF16, tag="zsrc")
            nc.gpsimd.memset(zsrc[:], 1.0)
            zacc = wm.tile([128, NS], F32, tag="zacc", name="zacc")
            for _ in range(N_WARM):
                nc.tensor.matmul(zacc[:], zsrc[:, 0:128], zsrc[:], start=True, stop=True)

            # ring A (sync): w-lo chunks interleaved with x0 halves, then x1..x7
            w8 = [
                w8p.tile([128, nkb, NS], I8, tag=f"w8_{ci}", name=f"w8_{ci}")
                for ci, nkb in enumerate(WCHUNKS)
            ]
            x8a = x8p.tile([128, 8, 128], I8, tag="x0a", name="x0a")
            x8b = x8p.tile([128, 8, 128], I8, tag="x0b", name="x0b")
            x8 = [
                x8p.tile([128, KT, 128], I8, tag=f"x8_{m}", name=f"x8_{m}")
                for m in range(1, MT)
            ]
            nc.sync.dma_start(w8[0][:], qw_in[:, 0:1, :])
            nc.sync.dma_start(w8[1][:], qw_in[:, 1:2, :])
            nc.sync.dma_start(w8[2][:], qw_in[:, 2:4, :])
            nc.sync.dma_start(x8a[:], qx_in[0][:, 0:8, :])
            nc.sync.dma_start(w8[3][:], qw_in[:, 4:6, :])
            nc.sync.dma_start(x8b[:], qx_in[0][:, 8:16, :])
            nc.sync.dma_start(w8[4][:], qw_in[:, 6:8, :])
            for m in range(1, MT):
                nc.sync.dma_start(x8[m - 1][:], qx_in[m])

            # ring B (scalar): w-hi bf16, 2 kb per trigger, straight to SBUF
            whi = [
                wbp.tile([128, 2, NS], BF16, tag=f"wh_{i}", name=f"wh_{i}")
                for i in range(4)
            ]
            for i in range(4):
                nc.scalar.dma_start(whi[i][:], wh_in[:, 2 * i : 2 * i + 2, :])

            # constants via SWDGE (gpsimd)
            scl_row = const.tile([1, 4], F32, tag="scl_row")
            nc.gpsimd.dma_start(scl_row[:], scl_in[:])
            sclb = const.tile([128, 4], F32, tag="sclb")
            nc.gpsimd.partition_broadcast(sclb[:], scl_row[:], channels=128)
            s_ap = sclb[:, 0:1]

            bias_row = const.tile([1, NS], F32, tag="bias_row")
            nc.gpsimd.dma_start(bias_row[:], b_in[:])
            bias_b = const.tile([128, NS], F32, tag="bias_b")
            nc.gpsimd.partition_broadcast(bias_b[:], bias_row[:], channels=128)

            # w-lo casts on DVE, one per chunk (arrival-paced)
            wlo = [
                wbp.tile([128, nkb, NS], BF16, tag=f"wl_{ci}", name=f"wl_{ci}")
                for ci, nkb in enumerate(WCHUNKS)
            ]
            for ci in range(len(WCHUNKS)):
                nc.vector.tensor_copy(wlo[ci][:], w8[ci][:])

            # x0 casts on ACT in arrival-order groups
            xb0 = xbp.tile([128, KT, 128], BF16, tag="xb0", name="xb0")
            for lo, hi, src, s0 in (
                (0, 2, x8a, 0), (2, 4, x8a, 2), (4, 8, x8a, 4),
                (8, 12, x8b, 0), (12, 16, x8b, 4),
            ):
                nc.scalar.activation(
                    xb0[:, lo:hi, :], src[:, s0 : s0 + hi - lo, :],
                    mybir.ActivationFunctionType.Copy,
                )

            # x1..x7 casts: DVE takes kb0-7, ACT kb8-15
            xb = [
                xbp.tile([128, KT, 128], BF16, tag=f"xb_{m}", name=f"xb_{m}")
                for m in range(1, MT)
            ]
            for m in range(1, MT):
                nc.vector.tensor_copy(xb[m - 1][:, 0:8, :], x8[m - 1][:, 0:8, :])
                nc.scalar.activation(
                    xb[m - 1][:, 8:16, :], x8[m - 1][:, 8:16, :],
                    mybir.ActivationFunctionType.Copy,
                )

            def w_ap(kb):
                if kb < KLO:
                    ci = int(np.searchsorted(wch_off, kb, side="right")) - 1
                    return wlo[ci][:, kb - wch_off[ci], :]
                return whi[(kb - KLO) // 2][:, (kb - KLO) % 2, :]

            def x_ap(mb, kb):
                if mb == 0:
                    return xb0[:, kb, :]
                return xb[mb - 1][:, kb, :]

            for mb in range(MT):
                acc = ps.tile([128, NS], F32, tag="acc")
                for kb in range(KT):
                    nc.tensor.matmul(
                        acc[:],
                        x_ap(mb, kb),
                        w_ap(kb),
                        start=(kb == 0),
                        stop=(kb == KT - 1),
                    )
                # fused dequant (acc*s + bias) -> SBUF in column chunks, each
                # chunk DMA'd immediately; last tile split finer across rings.
                o2 = op.tile([128, NS], F32, tag="o2")
                rows = out_t[mb * 128 : (mb + 1) * 128, :]
                nch = 4 if mb == MT - 1 else 2
                cw = NS // nch
                for c in range(nch):
                    sl = slice(c * cw, (c + 1) * cw)
                    nc.vector.scalar_tensor_tensor(
                        o2[:, sl], acc[:, sl], s_ap, bias_b[:, sl],
                        op0=mybir.AluOpType.mult, op1=mybir.AluOpType.add,
                    )
                    eng = nc.scalar if (mb < MT - 1 or c % 2 == 1) else nc.sync
                    eng.dma_start(rows[:, sl], o2[:, sl])

    nc.compile()
    return nc


_NC = None


def _get_nc():
    global _NC
    if _NC is None:
        _NC = _build_program()
    return _NC


def _quantize(a):
    """Exactly the reference's quantization: scale = amax/127 (f32 IEEE),
    q = clip(round-half-even(a / scale), -127, 127)."""
    amax = np.float32(np.max(np.abs(a)))
    scale = amax / np.float32(127.0)
    q = np.clip(np.round((a / scale).astype(np.float32)), -127.0, 127.0)
    return q.astype(np.int8), scale


def kernel(x, weight, bias, _trace=False):
    x = np.asarray(x, dtype=np.float32)
    weight = np.asarray(weight, dtype=np.float32)
    bias = np.asarray(bias, dtype=np.float32)

    qx, sx = _quantize(x)
    qw, sw = _quantize(weight)
    s = sx * sw
    scl = np.array([[s, sx, sw, 0.0]], dtype=np.float32)

    qxt = qx.T  # [K, M]
    qwt = qw.T  # [K, N]
    # k-lane-major w [k%128, k//128, n]
    wkl = np.ascontiguousarray(qwt.reshape(KT, 128, N).transpose(1, 0, 2))

    in_maps = []
    for c in range(8):
        i, j = divmod(c, PN)
        xs = qxt[:, i * MS : (i + 1) * MS]
        xs = np.ascontiguousarray(
            xs.reshape(KT, 128, MT, 128).transpose(2, 1, 0, 3)
        )  # [MT, 128, KT, 128]
        wc = wkl[:, :, j * NS : (j + 1) * NS]  # [128, KT, NS]
        in_maps.append(
            {
                "qxt_sh": xs,
                "qw_lo": np.ascontiguousarray(wc[:, :KLO, :]),
                "w_hi": np.ascontiguousarray(wc[:, KLO:, :]).astype(ml_dtypes.bfloat16),
                "b_sh": bias[j * NS : (j + 1) * NS].reshape(1, NS),
                "scl": scl,
            }
        )

    nc = _get_nc()
    try:
        res = run_bass_kernel_spmd(nc, in_maps, core_ids=list(range(8)), trace=_trace)
    except Exception:
        # rare transient NRT device hiccups recover on retry
        res = run_bass_kernel_spmd(nc, in_maps, core_ids=list(range(8)), trace=_trace)

    out = np.empty((M, N), np.float32)
    for c in range(8):
        i, j = divmod(c, PN)
        out[i * MS : (i + 1) * MS, j * NS : (j + 1) * NS] = res.results[c]["out_sh"]
    if _trace:
        return out, res
    return out


# revision 7
# speedup vs baseline: 1.0964x; 1.0321x over previous
"""Trainium2 kernel for nn_LinearMem: bit-sliced int8-quantized linear layer.

Math: the reference's 16 bit-plane matmuls recombine exactly to
qx @ qw^T with qx = round(x/sx), qw = round(w/sw).  Products/partials are
integers < 2^24, so bf16 x bf16 matmuls with f32 PSUM accumulation
reproduce the reference bitwise (int8 values are exact in bf16).

v3 schedule (per core; 2x4 tensor-parallel grid):
  - Input pipe analysis: one HWDGE ring sustains ~273 B/ns and the
    int8->bf16 cast engines ~219 (DVE) / ~125 (ACT) elem/ns, so m-tile 0
    (which consumes ALL of w) is paced by w delivery.  Fix: w k-blocks
    0-7 ship int8 on ring A and are DVE-cast; k-blocks 8-15 ship as
    bf16 (exact, values <= 127) on the otherwise-idle ring B, needing
    no cast and no ring-A bandwidth.
  - Ring A (sync): ramped w chunks [1,1,2,2,2] kb interleaved with the
    first x half-tiles, then x m1..m7 (int8, ACT+DVE cast halves).
  - 5 warmup matmuls release the HAM clock gate; real matmuls start
    ~10 us, m-tile 0 is input-paced to ~13.5 us, m1-m7 run at the
    216 ns/matmul roofline.
  - Output: per m-tile (acc*s + bias) in two 256-col DVE passes, each
    DMA'd immediately on ring B; the last tile uses four 128-col
    chunks on alternating rings so the final receipt chain is short.
"""

import sys

if "/opt/trn_rl_repo" not in sys.path:
    sys.path.insert(0, "/opt/trn_rl_repo")

import ml_dtypes
import numpy as np

import concourse.bacc as bacc
import concourse.mybir as mybir
import concourse.tile as tile
from concourse.bass_utils import run_bass_kernel_spmd

M, K, N = 2048, 2048, 2048
PM, PN = 2, 4
MS, NS = M // PM, N // PN  # 1024, 512
MT = MS // 128  # 8
KT = K // 128  # 16
KLO = 8  # k-blocks shipped int8 on ring A (DVE-cast); rest bf16 on ring B

F32 = mybir.dt.float32
BF16 = mybir.dt.bfloat16
I8 = mybir.dt.int8

# ring-A w chunk sizes in k-blocks (kb 0..7).  HWDGE constraints: ~8 DMA
# semaphores per ring before triggers serialize on reuse-waits, and
# per-partition lines must be >= 2048 B for line rate -> 2 chunks of 4 kb.
WCHUNKS = (4, 4)
N_WARM = 7


def _build_program():
    nc = bacc.Bacc("TRN2", target_bir_lowering=False, debug=False, num_devices=8)

    wch_off = np.concatenate([[0], np.cumsum(WCHUNKS)])
    assert wch_off[-1] == KLO

    qw_in = nc.dram_tensor("qw_lo", [128, KLO, NS], I8, kind="ExternalInput")
    wh_in = nc.dram_tensor("w_hi", [128, KT - KLO, NS], BF16, kind="ExternalInput")
    qx_in = nc.dram_tensor("qxt_sh", [MT, 128, KT, 128], I8, kind="ExternalInput")
    b_in = nc.dram_tensor("b_sh", [1, NS], F32, kind="ExternalInput")
    scl_in = nc.dram_tensor("scl", [1, 4], F32, kind="ExternalInput")
    out_t = nc.dram_tensor("out_sh", [MS, NS], F32, kind="ExternalOutput")

    with tile.TileContext(nc) as tc:
        with (
            tc.tile_pool(name="const", bufs=1) as const,
            tc.tile_pool(name="w8p", bufs=1) as w8p,
            tc.tile_pool(name="wbp", bufs=1) as wbp,
            tc.tile_pool(name="x8p", bufs=1) as x8p,
            tc.tile_pool(name="xbp", bufs=1) as xbp,
            tc.tile_pool(name="out", bufs=3) as op,
            tc.tile_pool(name="psum", bufs=4, space="PSUM") as ps,
            tc.tile_pool(name="warm", bufs=1, space="PSUM") as wm,
        ):
            # PE warmup: nonzero bf16 source (zero MACs are power-gated and
            # don't count as HAM activity); memset on gpsimd so the Tensor
            # queue isn't gated on the Vector preamble.
            zsrc = const.tile([128, NS], BF16, tag="zsrc")
            nc.gpsimd.memset(zsrc[:], 1.0)
            zacc = wm.tile([128, NS], F32, tag="zacc", name="zacc")
            for _ in range(N_WARM):
                nc.tensor.matmul(zacc[:], zsrc[:, 0:128], zsrc[:], start=True, stop=True)

            # ring A (sync): w-lo chunk, x0, w-lo chunk, then x1..x7
            w8 = [
                w8p.tile([128, nkb, NS], I8, tag=f"w8_{ci}", name=f"w8_{ci}")
                for ci, nkb in enumerate(WCHUNKS)
            ]
            x8 = [
                x8p.tile([128, KT, 128], I8, tag=f"x8_{m}", name=f"x8_{m}")
                for m in range(MT)
            ]
            nc.sync.dma_start(w8[0][:], qw_in[:, 0:4, :])
            nc.sync.dma_start(x8[0][:], qx_in[0])
            nc.sync.dma_start(w8[1][:], qw_in[:, 4:8, :])
            for m in range(1, MT):
                nc.sync.dma_start(x8[m][:], qx_in[m])

            # ring B (scalar): w-hi bf16, 4 kb per trigger, straight to SBUF
            whi = [
                wbp.tile([128, 4, NS], BF16, tag=f"wh_{i}", name=f"wh_{i}")
                for i in range(2)
            ]
            for i in range(2):
                nc.scalar.dma_start(whi[i][:], wh_in[:, 4 * i : 4 * i + 4, :])

            # constants via SWDGE (gpsimd)
            scl_row = const.tile([1, 4], F32, tag="scl_row")
            nc.gpsimd.dma_start(scl_row[:], scl_in[:])
            sclb = const.tile([128, 4], F32, tag="sclb")
            nc.gpsimd.partition_broadcast(sclb[:], scl_row[:], channels=128)
            s_ap = sclb[:, 0:1]

            bias_row = const.tile([1, NS], F32, tag="bias_row")
            nc.gpsimd.dma_start(bias_row[:], b_in[:])
            bias_b = const.tile([128, NS], F32, tag="bias_b")
            nc.gpsimd.partition_broadcast(bias_b[:], bias_row[:], channels=128)

            # casts, in per-queue emission order (queues execute in order):
            #   DVE:  w0[0:2], w0[2:4], x0[0:4], w1[0:2], w1[2:4],
            #         then x1..x7 kb0-7 halves
            #   ACT:  x0[4:10], x0[10:16], then x1..x7 kb8-15 halves
            wlo = [
                wbp.tile([128, 4, NS], BF16, tag=f"wl_{ci}", name=f"wl_{ci}")
                for ci in range(2)
            ]
            xb = [
                xbp.tile([128, KT, 128], BF16, tag=f"xb_{m}", name=f"xb_{m}")
                for m in range(MT)
            ]
            nc.vector.tensor_copy(wlo[0][:, 0:2, :], w8[0][:, 0:2, :])
            nc.vector.tensor_copy(wlo[0][:, 2:4, :], w8[0][:, 2:4, :])
            nc.vector.tensor_copy(xb[0][:, 0:4, :], x8[0][:, 0:4, :])
            nc.vector.tensor_copy(wlo[1][:, 0:2, :], w8[1][:, 0:2, :])
            nc.vector.tensor_copy(wlo[1][:, 2:4, :], w8[1][:, 2:4, :])
            nc.scalar.activation(
                xb[0][:, 4:10, :], x8[0][:, 4:10, :],
                mybir.ActivationFunctionType.Copy,
            )
            nc.scalar.activation(
                xb[0][:, 10:16, :], x8[0][:, 10:16, :],
                mybir.ActivationFunctionType.Copy,
            )
            for m in range(1, MT):
                nc.vector.tensor_copy(xb[m][:, 0:8, :], x8[m][:, 0:8, :])
                nc.scalar.activation(
                    xb[m][:, 8:16, :], x8[m][:, 8:16, :],
                    mybir.ActivationFunctionType.Copy,
                )

            def w_ap(kb):
                if kb < KLO:
                    return wlo[kb // 4][:, kb % 4, :]
                return whi[(kb - KLO) // 4][:, (kb - KLO) % 4, :]

            def x_ap(mb, kb):
                return xb[mb][:, kb, :]

            for mb in range(MT):
                acc = ps.tile([128, NS], F32, tag="acc")
                for kb in range(KT):
                    nc.tensor.matmul(
                        acc[:],
                        x_ap(mb, kb),
                        w_ap(kb),
                        start=(kb == 0),
                        stop=(kb == KT - 1),
                    )
                # fused dequant (acc*s + bias) -> SBUF in column chunks, each
                # chunk DMA'd immediately; last tile split finer across rings.
                o2 = op.tile([128, NS], F32, tag="o2")
                rows = out_t[mb * 128 : (mb + 1) * 128, :]
                nch = 4 if mb == MT - 1 else 2
                cw = NS // nch
                for c in range(nch):
                    sl = slice(c * cw, (c + 1) * cw)
                    nc.vector.scalar_tensor_tensor(
                        o2[:, sl], acc[:, sl], s_ap, bias_b[:, sl],
                        op0=mybir.AluOpType.mult, op1=mybir.AluOpType.add,
                    )
                    eng = nc.scalar if (mb < MT - 1 or c % 2 == 1) else nc.sync
                    eng.dma_start(rows[:, sl], o2[:, sl])

    nc.compile()
    return nc


_NC = None


def _get_nc():
    global _NC
    if _NC is None:
        _NC = _build_program()
    return _NC


def _quantize(a):
    """Exactly the reference's quantization: scale = amax/127 (f32 IEEE),
    q = clip(round-half-even(a / scale), -127, 127)."""
    amax = np.float32(np.max(np.abs(a)))
    scale = amax / np.float32(127.0)
    q = np.clip(np.round((a / scale).astype(np.float32)), -127.0, 127.0)
    return q.astype(np.int8), scale


def kernel(x, weight, bias, _trace=False):
    x = np.asarray(x, dtype=np.float32)
    weight = np.asarray(weight, dtype=np.float32)
    bias = np.asarray(bias, dtype=np.float32)

    qx, sx = _quantize(x)
    qw, sw = _quantize(weight)
    s = sx * sw
    scl = np.array([[s, sx, sw, 0.0]], dtype=np.float32)

    qxt = qx.T  # [K, M]
    qwt = qw.T  # [K, N]
    # k-lane-major w [k%128, k//128, n]
    wkl = np.ascontiguousarray(qwt.reshape(KT, 128, N).transpose(1, 0, 2))

    in_maps = []
    for c in range(8):
        i, j = divmod(c, PN)
        xs = qxt[:, i * MS : (i + 1) * MS]
        xs = np.ascontiguousarray(
            xs.reshape(KT, 128, MT, 128).transpose(2, 1, 0, 3)
        )  # [MT, 128, KT, 128]
        wc = wkl[:, :, j * NS : (j + 1) * NS]  # [128, KT, NS]
        in_maps.append(
            {
                "qxt_sh": xs,
                "qw_lo": np.ascontiguousarray(wc[:, :KLO, :]),
                "w_hi": np.ascontiguousarray(wc[:, KLO:, :]).astype(ml_dtypes.bfloat16),
                "b_sh": bias[j * NS : (j + 1) * NS].reshape(1, NS),
                "scl": scl,
            }
        )

    nc = _get_nc()
    try:
        res = run_bass_kernel_spmd(nc, in_maps, core_ids=list(range(8)), trace=_trace)
    except Exception:
        # rare transient NRT device hiccups recover on retry
        res = run_bass_kernel_spmd(nc, in_maps, core_ids=list(range(8)), trace=_trace)

    out = np.empty((M, N), np.float32)
    for c in range(8):
        i, j = divmod(c, PN)
        out[i * MS : (i + 1) * MS, j * NS : (j + 1) * NS] = res.results[c]["out_sh"]
    if _trace:
        return out, res
    return out


# revision 8
# speedup vs baseline: 1.1412x; 1.0409x over previous
"""Trainium2 kernel for nn_LinearMem: bit-sliced int8-quantized linear layer.

Math: the reference's 16 bit-plane matmuls recombine exactly to
qx @ qw^T with qx = round(x/sx), qw = round(w/sw).  Products/partials are
integers < 2^24, so bf16 x bf16 matmuls with f32 PSUM accumulation
reproduce the reference bitwise (int8 values are exact in bf16).

v5 schedule (per core; 2x4 tensor-parallel grid).  Measured HW facts this
is built around: one HWDGE ring sustains ~273 B/ns only while the other
ring is quiet (aggregate ~300), each ring has ~8 DMA semaphores before
triggers serialize on reuse-waits, per-partition DMA lines must be
>= 2048 B for line rate, DVE casts ~219 elem/ns and ACT ~125, and the
PE clock ramps 1.2->2.4 GHz only after ~4 us of sustained activity.

  - ring A (sync) carries ALL inputs int8, in arrival-greedy order
    [w0a 4kb, x0a 8kb, w0b 4kb, w1a 4kb, x0b 8kb, w1b 4kb, x1..x7];
    ring B (scalar) carries only outputs, so the input stream runs at
    the full solo-ring rate through the critical first ~13 us.
  - casts are emitted in data-arrival order: DVE does w at 2-k-block
    granularity, ACT does x0 halves, then both split x1..x7.
  - 7 warmup matmuls cover the clock-gate ramp; real matmuls start
    ~10.3 us, m-tile 0 is cast-paced to ~15 us, m1-m7 run at the
    216 ns/matmul roofline.
  - output: per m-tile fused dequant (acc*s + bias) in two 256-col DVE
    passes, each DMA'd immediately on ring B; the last tile uses four
    128-col chunks alternating rings so the final receipt chain is
    short.
"""

import sys

if "/opt/trn_rl_repo" not in sys.path:
    sys.path.insert(0, "/opt/trn_rl_repo")

import ml_dtypes
import numpy as np

import concourse.bacc as bacc
import concourse.mybir as mybir
import concourse.tile as tile
from concourse.bass_utils import run_bass_kernel_spmd

M, K, N = 2048, 2048, 2048
PM, PN = 2, 4
MS, NS = M // PM, N // PN  # 1024, 512
MT = MS // 128  # 8
KT = K // 128  # 16

F32 = mybir.dt.float32
BF16 = mybir.dt.bfloat16
I8 = mybir.dt.int8

N_WARM = 7


def _build_program():
    nc = bacc.Bacc("TRN2", target_bir_lowering=False, debug=False, num_devices=8)

    qw_in = nc.dram_tensor("qwt_sh", [128, KT, NS], I8, kind="ExternalInput")
    qx_in = nc.dram_tensor("qxt_sh", [MT, 128, KT, 128], I8, kind="ExternalInput")
    b_in = nc.dram_tensor("b_sh", [1, NS], F32, kind="ExternalInput")
    scl_in = nc.dram_tensor("scl", [1, 4], F32, kind="ExternalInput")
    out_t = nc.dram_tensor("out_sh", [MS, NS], F32, kind="ExternalOutput")

    with tile.TileContext(nc) as tc:
        with (
            tc.tile_pool(name="const", bufs=1) as const,
            tc.tile_pool(name="w8p", bufs=1) as w8p,
            tc.tile_pool(name="wbp", bufs=1) as wbp,
            tc.tile_pool(name="x8p", bufs=1) as x8p,
            tc.tile_pool(name="xbp", bufs=1) as xbp,
            tc.tile_pool(name="out", bufs=2) as op,
            tc.tile_pool(name="psum", bufs=4, space="PSUM") as ps,
            tc.tile_pool(name="warm", bufs=1, space="PSUM") as wm,
        ):
            # PE warmup: nonzero bf16 source (zero MACs are power-gated, no
            # HAM credit); memset on gpsimd so the Tensor queue isn't gated
            # on the Vector preamble.
            zsrc = const.tile([128, NS], BF16, tag="zsrc")
            nc.gpsimd.memset(zsrc[:], 1.0)
            zacc = wm.tile([128, NS], F32, tag="zacc", name="zacc")
            for _ in range(N_WARM):
                nc.tensor.matmul(zacc[:], zsrc[:, 0:128], zsrc[:], start=True, stop=True)

            # ring A (sync): all inputs, arrival-greedy interleave.
            # w quarters are 4 k-blocks [128, 4, 512] (2048 B lines); x0 in
            # two 8-k-block halves; x1..x7 whole (2048 B lines).
            w8 = [
                w8p.tile([128, 4, NS], I8, tag=f"w8_{q}", name=f"w8_{q}")
                for q in range(4)
            ]
            x0h = [
                x8p.tile([128, 8, 128], I8, tag=f"x0{h}", name=f"x0{h}")
                for h in range(2)
            ]
            x8 = [
                x8p.tile([128, KT, 128], I8, tag=f"x8_{m}", name=f"x8_{m}")
                for m in range(1, MT)
            ]
            nc.sync.dma_start(w8[0][:], qw_in[:, 0:4, :])
            nc.sync.dma_start(x0h[0][:], qx_in[0][:, 0:8, :])
            nc.sync.dma_start(w8[1][:], qw_in[:, 4:8, :])
            nc.sync.dma_start(w8[2][:], qw_in[:, 8:12, :])
            nc.sync.dma_start(x0h[1][:], qx_in[0][:, 8:16, :])
            nc.sync.dma_start(w8[3][:], qw_in[:, 12:16, :])
            for m in range(1, MT):
                nc.sync.dma_start(x8[m - 1][:], qx_in[m])

            # constants via SWDGE (gpsimd)
            scl_row = const.tile([1, 4], F32, tag="scl_row")
            nc.gpsimd.dma_start(scl_row[:], scl_in[:])
            sclb = const.tile([128, 4], F32, tag="sclb")
            nc.gpsimd.partition_broadcast(sclb[:], scl_row[:], channels=128)
            s_ap = sclb[:, 0:1]

            bias_row = const.tile([1, NS], F32, tag="bias_row")
            nc.gpsimd.dma_start(bias_row[:], b_in[:])
            bias_b = const.tile([128, NS], F32, tag="bias_b")
            nc.gpsimd.partition_broadcast(bias_b[:], bias_row[:], channels=128)

            # casts in data-arrival order.
            # DVE: w quarters at 2-kb granularity; ACT: x0 halves.
            wb = [
                wbp.tile([128, 4, NS], BF16, tag=f"wb_{q}", name=f"wb_{q}")
                for q in range(4)
            ]
            xb = [
                xbp.tile([128, KT, 128], BF16, tag=f"xb_{m}", name=f"xb_{m}")
                for m in range(MT)
            ]
            nc.vector.tensor_copy(wb[0][:, 0:2, :], w8[0][:, 0:2, :])
            nc.vector.tensor_copy(wb[0][:, 2:4, :], w8[0][:, 2:4, :])
            nc.scalar.activation(
                xb[0][:, 0:4, :], x0h[0][:, 0:4, :],
                mybir.ActivationFunctionType.Copy,
            )
            nc.scalar.activation(
                xb[0][:, 4:8, :], x0h[0][:, 4:8, :],
                mybir.ActivationFunctionType.Copy,
            )
            nc.vector.tensor_copy(wb[1][:, 0:2, :], w8[1][:, 0:2, :])
            nc.vector.tensor_copy(wb[1][:, 2:4, :], w8[1][:, 2:4, :])
            nc.vector.tensor_copy(wb[2][:, 0:2, :], w8[2][:, 0:2, :])
            nc.scalar.activation(
                xb[0][:, 8:12, :], x0h[1][:, 0:4, :],
                mybir.ActivationFunctionType.Copy,
            )
            nc.scalar.activation(
                xb[0][:, 12:16, :], x0h[1][:, 4:8, :],
                mybir.ActivationFunctionType.Copy,
            )
            nc.vector.tensor_copy(wb[2][:, 2:4, :], w8[2][:, 2:4, :])
            nc.vector.tensor_copy(wb[3][:, 0:2, :], w8[3][:, 0:2, :])
            nc.vector.tensor_copy(wb[3][:, 2:4, :], w8[3][:, 2:4, :])
            # x1..x7: DVE kb0-7, ACT kb8-15
            for m in range(1, MT):
                nc.vector.tensor_copy(xb[m][:, 0:8, :], x8[m - 1][:, 0:8, :])
                nc.scalar.activation(
                    xb[m][:, 8:16, :], x8[m - 1][:, 8:16, :],
                    mybir.ActivationFunctionType.Copy,
                )

            for mb in range(MT):
                acc = ps.tile([128, NS], F32, tag="acc")
                for kb in range(KT):
                    nc.tensor.matmul(
                        acc[:],
                        xb[mb][:, kb, :],
                        wb[kb // 4][:, kb % 4, :],
                        start=(kb == 0),
                        stop=(kb == KT - 1),
                    )
                # fused dequant (acc*s + bias) -> SBUF in column chunks, each
                # chunk DMA'd immediately on ring B; last tile split finer
                # across both rings.
                o2 = op.tile([128, NS], F32, tag="o2")
                rows = out_t[mb * 128 : (mb + 1) * 128, :]
                nch = 4 if mb == MT - 1 else 2
                cw = NS // nch
                for c in range(nch):
                    sl = slice(c * cw, (c + 1) * cw)
                    nc.vector.scalar_tensor_tensor(
                        o2[:, sl], acc[:, sl], s_ap, bias_b[:, sl],
                        op0=mybir.AluOpType.mult, op1=mybir.AluOpType.add,
                    )
                    eng = nc.scalar if (mb < MT - 1 or c % 2 == 1) else nc.sync
                    eng.dma_start(rows[:, sl], o2[:, sl])

    nc.compile()
    return nc


_NC = None


def _get_nc():
    global _NC
    if _NC is None:
        _NC = _build_program()
    return _NC


def _quantize(a):
    """Exactly the reference's quantization: scale = amax/127 (f32 IEEE),
    q = clip(round-half-even(a / scale), -127, 127)."""
    amax = np.float32(np.max(np.abs(a)))
    scale = amax / np.float32(127.0)
    q = np.clip(np.round((a / scale).astype(np.float32)), -127.0, 127.0)
    return q.astype(np.int8), scale


def kernel(x, weight, bias, _trace=False):
    x = np.asarray(x, dtype=np.float32)
    weight = np.asarray(weight, dtype=np.float32)
    bias = np.asarray(bias, dtype=np.float32)

    qx, sx = _quantize(x)
    qw, sw = _quantize(weight)
    s = sx * sw
    scl = np.array([[s, sx, sw, 0.0]], dtype=np.float32)

    qxt = qx.T  # [K, M]
    qwt = qw.T  # [K, N]
    # k-lane-major w [k%128, k//128, n]
    wkl = np.ascontiguousarray(qwt.reshape(KT, 128, N).transpose(1, 0, 2))

    in_maps = []
    for c in range(8):
        i, j = divmod(c, PN)
        xs = qxt[:, i * MS : (i + 1) * MS]
        xs = np.ascontiguousarray(
            xs.reshape(KT, 128, MT, 128).transpose(2, 1, 0, 3)
        )  # [MT, 128, KT, 128]
        in_maps.append(
            {
                "qxt_sh": xs,
                "qwt_sh": np.ascontiguousarray(wkl[:, :, j * NS : (j + 1) * NS]),
                "b_sh": bias[j * NS : (j + 1) * NS].reshape(1, NS),
                "scl": scl,
            }
        )

    nc = _get_nc()
    try:
        res = run_bass_kernel_spmd(nc, in_maps, core_ids=list(range(8)), trace=_trace)
    except Exception:
        # rare transient NRT device hiccups recover on retry
        res = run_bass_kernel_spmd(nc, in_maps, core_ids=list(range(8)), trace=_trace)

    out = np.empty((M, N), np.float32)
    for c in range(8):
        i, j = divmod(c, PN)
        out[i * MS : (i + 1) * MS, j * NS : (j + 1) * NS] = res.results[c]["out_sh"]
    if _trace:
        return out, res
    return out
